# revision 40
# baseline (speedup 1.0000x reference)
"""Trainium2 Bass kernel for a binarized (1w/1a) BasicBlock — fp8 DoubleRow.

    a1 = sign(x);  y1 = BN(conv3x3(a1, binarize(w1))) + x;  x1 = maxout(y1)
    a2 = sign(x1); y2 = BN(conv3x3(a2, binarize(w2))) + x1; out = maxout(y2)

Data-parallel over batch (4 samples/core, 8 cores); exact binary math:
activations are +-1 (fp8e4, exact), weights are sign(+-1) fp8; each conv is
9 DoubleRow matmuls per (chunk, cout-block), contracting all 256 input
channels at once over contiguous padded-row runs (pad columns land in
unused psum columns).  conv_true = alpha_a*alpha[o]*(BB + q[o]*S1) with
q = beta/alpha; S1 (3x3 box of the channel sum) comes from 3 more DoubleRow
ones-matmuls (folding the kh taps) + 2 shifted adds.  The per-channel scale
folds into BN exactly by scaling BN_EPS per channel.

Batch-stat BN: per (round, cout-block) AllReduce of (sum, sumsq) [128,2].
The conv loop is block-outer so block 0's AllReduce launches halfway
through the conv and overlaps the block-1 matmuls; one warmup AllGather is
triggered first-thing so the ~30us cold collective-firmware init runs
under conv1.

Maxout is sign-based: out = t * (sign(t)*(p-n)/2 + (p+n)/2), reusing the
sign values the next conv needs anyway.
"""

import numpy as np
import ml_dtypes

import concourse.bass as bass
import concourse.bacc as bacc
import concourse.mybir as mybir
import concourse.tile as tile

N_CORES = 8
B, C, H, W = 32, 256, 28, 28
BPC = B // N_CORES            # samples per core
NBLK = 2                      # channel blocks of 128
HPAD, WPAD = 30, 32           # padded image in SBUF
PIX = H * W                   # 784
PPIX = HPAD * WPAD            # 960
NCHUNK = 2 * BPC              # 8 chunks of (sample, half-image)
HHALF = H // 2                # 14
CHUNK = HHALF * W             # 392 dense output elems per chunk
RUN = HHALF * WPAD            # 448: rhs run length / psum width per chunk
BN_EPS = 1e-5
NPRM = 28
GUARD = 32                    # fp8 guard elems around merged activation tile
PLANE = BPC * PPIX            # 3840 elems per channel-block plane
SPAN = 4 * CHUNK              # 1568-wide apply spans (2 samples)
CNT = float(NCHUNK * CHUNK * N_CORES)   # 25088 elems per channel for BN
F32 = mybir.dt.float32
BF16 = mybir.dt.bfloat16
FP8 = mybir.dt.float8e4
AF = mybir.ActivationFunctionType
ALU = mybir.AluOpType
DR = mybir.MatmulPerfMode.DoubleRow
RG = [list(range(N_CORES))]


def _evac(nc, sc, ps, s1, sums, sumsqs, cv, prm, pcol, ci, oblk):
    """z = q[o]*S1 + BB from PSUM (strided: skip pad cols).  Stats come for
    free: the STT accumulates sum(z) on DVE; a Square pass on the otherwise
    idle ScalarE accumulates sum(z^2).  sums/sumsqs are per-block tiles so
    one block's stats never depend on the other block's evacuations."""
    psv = ps[:].rearrange("p (h w) -> p h w", h=HHALF)[:, :, 2:2 + W]
    s1v = s1[:].rearrange("p (h w) -> p h w", h=H)[
        :, (ci % 2) * HHALF:(ci % 2) * HHALF + HHALF, :]
    cvc = cv[oblk][:, ci * CHUNK:(ci + 1) * CHUNK]
    nc.vector.scalar_tensor_tensor(
        cvc.rearrange("p (h w) -> p h w", h=HHALF), s1v,
        prm[:, pcol['q'] + oblk:pcol['q'] + oblk + 1], psv,
        op0=ALU.mult, op1=ALU.add,
        accum_out=sums[oblk][:, ci:ci + 1])
    sqj = sc.tile([128, CHUNK], F32, tag="sqj", name="sqj", bufs=2)
    nc.scalar.activation(
        sqj[:], cvc, AF.Square,
        accum_out=sumsqs[oblk][:, ci:ci + 1])


def _loc_stats(nc, sc, oblk, sums, sumsqs, out2):
    """Reduce this block's per-chunk partials into local (sum, sumsq)."""
    nc.vector.reduce_sum(out2[:, 0:1], sums[oblk][:],
                         axis=mybir.AxisListType.X)
    nc.vector.reduce_sum(out2[:, 1:2], sumsqs[oblk][:],
                         axis=mybir.AxisListType.X)


def _stats_issue_ar4(nc, tc, pools, rnd, loc4):
    """One AllReduce of both blocks' (sum, sumsq); result via sync DMA."""
    sbuf, psum, sc, dram = pools
    with tc.high_priority():
        a_in = dram.tile([128, 4], F32, name=f"arin{rnd}")
        a_out = dram.tile([128, 4], F32, name=f"arout{rnd}")
        nc.sync.dma_start(a_in[:], loc4[:])
        nc.gpsimd.collective_compute(
            "AllReduce", ALU.add, replica_groups=RG,
            ins=[a_in.opt()], outs=[a_out.opt()])
        tot = sbuf.tile([128, 4], F32, name=f"tot{rnd}")
        nc.sync.dma_start(tot[:], a_out[:])
    return tot


def _stats_issue_ag(nc, tc, pools, rnd, sums, sumsqs):
    """One 2-phase AllGather of both blocks' (sum, sumsq) pairs; the
    cross-rank reduction happens on DVE after gather.  Used for round 1,
    where this is the first collective and eats the launch skew."""
    sbuf, psum, sc, dram = pools
    with tc.high_priority():
        loc4 = sbuf.tile([128, 4], F32, name=f"loc4_{rnd}")
        _loc_stats(nc, sc, 0, sums, sumsqs, loc4[:, 0:2])
        _loc_stats(nc, sc, 1, sums, sumsqs, loc4[:, 2:4])
        a_in = dram.tile([128, 4], F32, name=f"agin{rnd}")
        a_out = dram.tile([N_CORES, 128, 4], F32, name=f"agout{rnd}")
        nc.sync.dma_start(a_in[:], loc4[:])
        nc.gpsimd.collective_compute(
            "AllGather", ALU.bypass, replica_groups=RG,
            ins=[a_in.opt()], outs=[a_out.opt()])
        gst = sbuf.tile([128, N_CORES * 4], F32, name=f"gst{rnd}")
        gst_dma = nc.sync.dma_start(
            gst[:].rearrange("p (r j) -> p r j", r=N_CORES),
            a_out[:].rearrange("r p j -> p r j"))
        tot4 = sbuf.tile([128, 4], F32, name=f"tot4_{rnd}")
        nc.vector.reduce_sum(tot4[:],
                             gst[:].rearrange("p (r j) -> p j r", r=N_CORES),
                             axis=mybir.AxisListType.X)
    return tot4, gst_dma


def _stats_fin(nc, pools, rnd, oblk, tot2, prm, pcol):
    """Global (sum, sumsq) AP -> BN scale/shift columns for this block."""
    sbuf, psum, sc, dram = pools
    mm = sbuf.tile([128, 2], F32, name=f"mm{rnd}_{oblk}")
    m2 = sbuf.tile([128, 1], F32, name=f"m2{rnd}_{oblk}")
    var = sbuf.tile([128, 1], F32, name=f"var{rnd}_{oblk}")
    sd = sbuf.tile([128, 1], F32, name=f"sd{rnd}_{oblk}")
    inv = sbuf.tile([128, 1], F32, name=f"inv{rnd}_{oblk}")
    scale = sbuf.tile([128, 1], F32, name=f"scale{rnd}_{oblk}")
    tmp = sbuf.tile([128, 1], F32, name=f"tmp{rnd}_{oblk}")
    shift = sbuf.tile([128, 1], F32, name=f"shift{rnd}_{oblk}")
    nc.vector.tensor_scalar_mul(mm[:], tot2, 1.0 / CNT)
    nc.vector.tensor_mul(m2[:], mm[:, 0:1], mm[:, 0:1])
    nc.vector.tensor_sub(var[:], mm[:, 1:2], m2[:])
    nc.scalar.activation(sd[:], var[:], AF.Sqrt,
                         bias=prm[:, pcol['eps'] + oblk:
                                  pcol['eps'] + oblk + 1],
                         scale=1.0)
    nc.vector.reciprocal(inv[:], sd[:])
    nc.vector.tensor_mul(scale[:], inv[:],
                         prm[:, pcol['g'] + oblk:pcol['g'] + oblk + 1])
    nc.vector.tensor_mul(tmp[:], mm[:, 0:1], scale[:])
    nc.vector.tensor_sub(shift[:],
                         prm[:, pcol['b'] + oblk:pcol['b'] + oblk + 1],
                         tmp[:])
    return scale, shift


class Round:
    """Issue helpers for one conv+BN+residual+maxout round.

    build() drives the two rounds as one interleaved issue sequence —
    round 2's matmul chunks and evacuations are issued BETWEEN round 1's
    apply span groups so each engine's static queue order alternates
    between pacing the next conv and draining the stats chain.
    """

    def __init__(self, nc, tc, pools, rnd, apad, wsb, xres, x1b, cv, prm,
                 pcol, onesb, out_d=None, a2pad=None):
        self.__dict__.update(locals())
        sbuf, psum, sc, dram = pools
        self.sc, self.psum = sc, psum
        self.sums = [sbuf.tile([128, NCHUNK], F32, name=f"sums{rnd}_{i}")
                     for i in range(NBLK)]
        self.sumsqs = [sbuf.tile([128, NCHUNK], F32, name=f"sumsqs{rnd}_{i}")
                       for i in range(NBLK)]
        # sample-major activation layout (b, i, h, w): the rhs AP's
        # contiguous dependency footprint (PPIX + RUN wide) stays inside
        # one sample, so each sample's matmuls depend only on that
        # sample's sign writes
        self.t448 = apad[:, 0:2 * PPIX].rearrange(
            "p (i n) -> p i n", i=2)[:, :, 0:RUN]
        self.wv = wsb[:].rearrange("p (k i o) -> p k i o", k=9, i=NBLK)
        self.ones3 = onesb[:].rearrange("p (i o) -> p i o", i=2)
        self.s1s = {}
        self.ss = {}
        self.tots = {}
        self.pend_muls = []
        if a2pad is not None:
            self.a2v = a2pad[:, GUARD:GUARD + 2 * PLANE].rearrange(
                "p (b i h w) -> p b i h w", b=BPC, i=2, h=HPAD, w=WPAD)
        if out_d is not None:
            self.ov = out_d[:].rearrange("(bp b2) c h w -> c bp b2 (h w)",
                                         bp=2)

    def rhs_ap(self, b, off):
        return bass.AP(self.t448.tensor, GUARD + b * 2 * PPIX + off,
                       self.t448.ap)

    def s1_build(self, b):
        nc, sc, psum, rnd = self.nc, self.sc, self.psum, self.rnd
        hs = sc.tile([128, 2 * RUN], F32, tag="hs", name="hs", bufs=2)
        for half in range(2):
            h0 = half * HHALF
            ps2 = psum.tile([128, RUN], F32, tag="ps2", name=f"ps2_{rnd}",
                            bufs=2)
            for kh in range(3):
                nc.tensor.matmul(ps2[:], self.ones3,
                                 self.rhs_ap(b, (h0 + kh) * WPAD),
                                 start=(kh == 0), stop=(kh == 2),
                                 perf_mode=DR)
            nc.scalar.copy(hs[:, half * RUN:half * RUN + RUN], ps2[:])
        # W-direction 3-tap over the whole sample (28 rows x 32)
        hsv = hs[:].rearrange("p (h w) -> p h w", h=H)
        w3 = sc.tile([128, H * W], F32, tag="w3", name="w3", bufs=2)
        w3v = w3[:].rearrange("p (h w) -> p h w", h=H)
        nc.gpsimd.tensor_add(w3v, hsv[:, :, 1:1 + W], hsv[:, :, 2:2 + W])
        s1 = sc.tile([128, H * W], F32, tag="s1", name="s1", bufs=BPC)
        s1v = s1[:].rearrange("p (h w) -> p h w", h=H)
        nc.vector.tensor_add(s1v, w3v, hsv[:, :, 3:3 + W])
        self.s1s[b] = s1

    def chunk(self, ci, oblk):
        nc, rnd = self.nc, self.rnd
        b, h0 = ci // 2, (ci % 2) * HHALF
        ps = self.psum.tile([128, RUN], F32, tag="ps", name=f"ps{rnd}",
                            bufs=6)
        for k9 in range(9):
            kh, kw = k9 // 3, k9 % 3
            nc.tensor.matmul(
                ps[:], self.wv[:, k9, :, oblk * 128:(oblk + 1) * 128],
                self.rhs_ap(b, (h0 + kh) * WPAD + kw - 1),
                start=(k9 == 0), stop=(k9 == 8), perf_mode=DR)
        _evac(nc, self.sc, ps, self.s1s[b], self.sums, self.sumsqs,
              self.cv, self.prm, self.pcol, ci, oblk)

    def stats_ag(self):
        self.tot4, self.gst_dma = _stats_issue_ag(
            self.nc, self.tc, self.pools, self.rnd, self.sums, self.sumsqs)

    def loc_block(self, oblk):
        """Reduce one block's partials into the shared loc4 tile (on the
        gpsimd queue, as early as that block's chunks are done)."""
        if not hasattr(self, "loc4"):
            sbuf = self.pools[0]
            self.loc4 = sbuf.tile([128, 4], F32, name=f"loc4r{self.rnd}")
        _loc_stats(self.nc, self.sc, oblk, self.sums, self.sumsqs,
                   self.loc4[:, 2 * oblk:2 * oblk + 2])

    def stats_ar(self):
        self.tot4 = _stats_issue_ar4(self.nc, self.tc, self.pools,
                                     self.rnd, self.loc4)

    def fin(self, oblk):
        self.ss[oblk] = _stats_fin(self.nc, self.pools, self.rnd, oblk,
                                   self.tot4[:, 2 * oblk:2 * oblk + 2],
                                   self.prm, self.pcol)

    def flush_muls(self):
        for oblk, lo, n, t, coef in self.pend_muls:
            self.nc.vector.tensor_mul(self.x1b[oblk][:, lo:lo + n],
                                      t[:], coef[:])
        self.pend_muls = []

    def apply_span(self, oblk, lo, n):
        """BN-apply + residual + maxout over cols [lo, lo+n).

        Round 1 stays fp32 (a2 = sign(t) feeds conv2 — bf16 rounding of t
        flips signs near zero, which cascades through the next conv).
        Round 2 runs bf16: its sign only scales a near-zero t, so flips
        are harmless, and the 16-bit DVE mode doubles tensor_tensor
        throughput.  x1 (the round-2 residual) is stored bf16."""
        nc, sc, prm, pcol = self.nc, self.sc, self.prm, self.pcol
        scale, shift = self.ss[oblk]
        cvs = self.cv[oblk][:, lo:lo + n]
        dt = F32 if self.a2pad is not None else BF16
        u = sc.tile([128, n], dt, tag="u", name="u", bufs=2)
        nc.vector.tensor_scalar(u[:], cvs, scale[:], shift[:],
                                op0=ALU.mult, op1=ALU.add)
        t = sc.tile([128, n], dt, tag="t", name="t", bufs=8)
        coef = sc.tile([128, n], dt, tag="coef", name="coef", bufs=8)
        if self.a2pad is not None:
            nc.vector.tensor_add(t[:], u[:], self.xres[oblk][:, lo:lo + n])
            # sign -> next conv's fp8 input (padded); coef read from it
            b = lo // PIX
            sg = self.a2v[:, b, oblk, 1:1 + H, 2:2 + W]
            nc.scalar.activation(sg, t[:].rearrange("p (h w) -> p h w", h=H),
                                 AF.Sign)
            nc.scalar.activation(
                coef[:].rearrange("p (h w) -> p h w", h=H),
                sg, AF.Identity,
                bias=prm[:, pcol['hs'] + oblk:pcol['hs'] + oblk + 1],
                scale=prm[:, pcol['hp'] + oblk:pcol['hp'] + oblk + 1])
            # x1 = t*coef feeds only round 2's apply — defer it (see
            # flush_muls) so the DVE paces the sign chain, not this
            self.pend_muls.append((oblk, lo, n, t, coef))
        else:
            nc.vector.tensor_add(t[:], u[:], self.x1b[oblk][:, lo:lo + n])
            sgt = sc.tile([128, n], BF16, tag="sg", name="sg", bufs=2)
            nc.scalar.activation(sgt[:], t[:], AF.Sign)
            # coef on DVE: bf16 tensor_scalar runs in the 4x mode
            nc.vector.tensor_scalar(
                coef[:], sgt[:],
                prm[:, pcol['hp'] + oblk:pcol['hp'] + oblk + 1],
                prm[:, pcol['hs'] + oblk:pcol['hs'] + oblk + 1],
                op0=ALU.mult, op1=ALU.add)
            och = sc.tile([128, n], BF16, tag="och", name="och", bufs=2)
            nc.vector.tensor_mul(och[:], t[:], coef[:])
            q = nc.gpsimd if oblk == 0 else nc.sync
            q.dma_start(self.ov[oblk * 128:oblk * 128 + 128, lo // SPAN],
                        och[:].rearrange("p (b2 hw) -> p b2 hw", b2=2))


def build():
    nc = bacc.Bacc("TRN2", target_bir_lowering=False, debug=False,
                   enable_asserts=True, num_devices=N_CORES)
    x_d = nc.dram_tensor("x", [BPC, C, H, W], F32, kind="ExternalInput")
    w1_d = nc.dram_tensor("w1t", [9, NBLK, 128, 256], FP8,
                          kind="ExternalInput")
    w2_d = nc.dram_tensor("w2t", [9, NBLK, 128, 256], FP8,
                          kind="ExternalInput")
    prm_d = nc.dram_tensor("prm", [128, NPRM], F32, kind="ExternalInput")
    out_d = nc.dram_tensor("out", [BPC, C, H, W], BF16,
                           kind="ExternalOutput")

    with tile.TileContext(nc) as tc:
        with (
            tc.tile_pool(name="sbuf", bufs=1) as sbuf,
            tc.tile_pool(name="psum", bufs=6, space="PSUM") as psum,
            tc.tile_pool(name="sc", bufs=2) as sc,
            tc.tile_pool(name="dram", bufs=1, space="DRAM") as dram,
        ):
            pools = (sbuf, psum, sc, dram)
            # warm up the collective stack first-thing: the ~30us cold ncfw
            # init starts at the TRIGGER, so get it out in the first ~1us
            # (memset on gpsimd's head, DMA + trigger on the idle sync queue)
            wu_i = dram.tile([1, 16], F32, name="wu_i")
            wu_o = dram.tile([N_CORES, 16], F32, name="wu_o")
            with tc.high_priority():
                # source the warmup payload straight from the x input in
                # DRAM: no compute dependency, so the trigger (which
                # starts the ~30us ncfw cold init) fires immediately
                nc.sync.dma_start(
                    wu_i[:],
                    x_d[:].rearrange("b c h w -> (b c h) w")[0:1, 0:16])
                nc.gpsimd.collective_compute(
                    "AllGather", ALU.bypass, replica_groups=RG,
                    ins=[wu_i.opt()], outs=[wu_o.opt()])

            w1sb = sbuf.tile([128, 9 * NBLK * 256], FP8, name="w1sb")
            w2sb = sbuf.tile([128, 9 * NBLK * 256], FP8, name="w2sb")
            prm = sbuf.tile([128, NPRM], F32, name="prm")
            onesb = sbuf.tile([128, 256], FP8, name="onesb")
            nc.vector.memset(onesb[:], 1.0)
            xres = [sbuf.tile([128, BPC * PIX], F32, name=f"xres{i}")
                    for i in range(NBLK)]
            x1b = [sbuf.tile([128, BPC * PIX], BF16, name=f"x1b{i}")
                   for i in range(NBLK)]
            a1p = sbuf.tile([128, GUARD + 2 * PLANE + GUARD], FP8, name="a1p")
            a2p = sbuf.tile([128, GUARD + 2 * PLANE + GUARD], FP8, name="a2p")
            cv = [sbuf.tile([128, BPC * PIX], F32, name=f"cv{i}")
                  for i in range(NBLK)]

            nc.vector.memset(a1p[:].bitcast(mybir.dt.uint32), 0)
            xv = x_d[:].rearrange("b c h w -> c b (h w)")
            for b in range(BPC):
                for i in range(NBLK):
                    nc.sync.dma_start(xres[i][:, b * PIX:(b + 1) * PIX],
                                      xv[i * 128:(i + 1) * 128, b])
            # w1 split across three DMA queues so no single 8us transfer
            # gates the first matmul
            w1v = w1sb[:].rearrange("p (k i o) -> p k i o", k=9, i=NBLK)
            w1dv = w1_d[:].rearrange("k i p o -> p k i o")
            nc.scalar.dma_start(w1v[:, 0:3], w1dv[:, 0:3])
            nc.gpsimd.dma_start(w1v[:, 3:6], w1dv[:, 3:6])
            nc.sync.dma_start(w1v[:, 6:9], w1dv[:, 6:9])
            nc.sync.dma_start(prm[:], prm_d[:])
            nc.gpsimd.memset(a2p[:].bitcast(mybir.dt.uint32), 0)
            nc.sync.dma_start(
                w2sb[:].rearrange("p (k i o) -> p k i o", k=9, i=NBLK),
                w2_d[:].rearrange("k i p o -> p k i o"))
            # a1 = sign(x) (+-1) into padded interior of merged fp8 tile
            a1v = a1p[:, GUARD:GUARD + 2 * PLANE].rearrange(
                "p (b i h w) -> p b i h w", b=BPC, i=2, h=HPAD, w=WPAD)
            xrvs = [xres[i][:].rearrange("p (b h w) -> p b h w", b=BPC, h=H)
                    for i in range(NBLK)]
            sg00 = None
            for b in range(BPC):
                for i in range(NBLK):
                    sg = nc.scalar.activation(a1v[:, b, i, 1:1 + H, 2:2 + W],
                                              xrvs[i][:, b], AF.Sign)
                    if sg00 is None:
                        sg00 = sg

            pcol1 = {'g': 0, 'b': 2, 'hp': 4, 'hs': 6, 'eps': 16, 'q': 20}
            pcol2 = {'g': 8, 'b': 10, 'hp': 12, 'hs': 14, 'eps': 18,
                     'q': 22, 'pm': 24, 'hn': 26}
            r1 = Round(nc, tc, pools, 1, a1p, w1sb, xres, x1b, cv, prm,
                       pcol1, onesb, a2pad=a2p)
            r2 = Round(nc, tc, pools, 2, a2p, w2sb, xres, x1b, cv, prm,
                       pcol2, onesb, out_d=out_d)
            # HAM warm-up: the PE idles >3.4us before each conv stream and
            # re-throttles to 1.2 GHz; ~10 dummy matmuls timed to end right
            # as the real stream starts keep the first ~30 real matmuls at
            # 2.4 GHz.  Round-1 dummies read the just-zeroed a2p (ready
            # ~2us before the first sign completes).
            for k in range(10):
                wps = psum.tile([128, RUN], F32, tag="ps2", name="warm1",
                                bufs=2)
                nc.tensor.matmul(wps[:], r2.ones3, r2.rhs_ap(0, k * WPAD),
                                 start=True, stop=True, perf_mode=DR)
            # bridge dummies gated on the first sign, so the PE can't
            # re-throttle in the jitter window before the real stream
            wg1 = None
            for k in range(4):
                wps = psum.tile([128, RUN], F32, tag="ps2", name="warm1b",
                                bufs=2)
                mm = nc.tensor.matmul(wps[:], r2.ones3,
                                      r2.rhs_ap(0, (10 + k) * WPAD),
                                      start=True, stop=True, perf_mode=DR)
                if wg1 is None:
                    wg1 = mm
            bass._add_dep_helper(wg1.ins, sg00.ins, sync=True,
                                 reason="ham-warm bridge on first sign")
            # round 1 conv + single skew-eating AllGather
            for b in range(BPC):
                r1.s1_build(b)
            for oblk in range(NBLK):
                for ci in range(NCHUNK):
                    r1.chunk(ci, oblk)
            r1.stats_ag()
            # round-2 warm-up dummies, gated on the AllGather result
            # arriving (~5us before the first real conv2 matmul); they
            # read round-1's stale a1p, which has no further writers
            wgate = None
            for k in range(10):
                wps = psum.tile([128, RUN], F32, tag="ps2", name="warm2",
                                bufs=2)
                mm = nc.tensor.matmul(wps[:], r1.ones3, r1.rhs_ap(0, k * WPAD),
                                      start=True, stop=True, perf_mode=DR)
                if wgate is None:
                    wgate = mm
            bass._add_dep_helper(wgate.ins, r1.gst_dma.ins, sync=True,
                                 reason="ham-warm gate on AG1 result")
            r1.fin(0)
            r1.fin(1)
            # apply-1 spans interleaved with round-2 conv issue so every
            # engine queue alternates between pacing conv2 and draining
            # the round-2 stats chain
            for b in (0, 1):
                for oblk in range(NBLK):
                    r1.apply_span(oblk, b * PIX, PIX)
            r2.s1_build(0)
            r2.s1_build(1)
            for ci in range(4):
                r2.chunk(ci, 0)
            for b in (2, 3):
                for oblk in range(NBLK):
                    r1.apply_span(oblk, b * PIX, PIX)
            r2.s1_build(2)
            r2.s1_build(3)
            r1.flush_muls()
            for ci in range(4, NCHUNK):
                r2.chunk(ci, 0)
            r2.loc_block(0)
            for ci in range(NCHUNK):
                r2.chunk(ci, 1)
            r2.loc_block(1)
            r2.stats_ar()
            r2.fin(0)
            r2.fin(1)
            for sp in range(BPC // 2):
                r2.apply_span(0, sp * SPAN, SPAN)
            for sp in range(BPC // 2):
                r2.apply_span(1, sp * SPAN, SPAN)

    nc.compile()
    return nc


def _prep_weight(w):
    """(O,I,3,3) fp32 -> sign lhsT (9, iblk, 128, 256) fp8 (+-1, exact),
    plus per-output-channel alpha, beta (float64)."""
    w = w.astype(np.float64)
    beta = w.mean(axis=(1, 2, 3))
    alpha = np.sqrt(((w - beta[:, None, None, None]) ** 2)
                    .mean(axis=(1, 2, 3)))
    s = np.sign(w - beta[:, None, None, None]).astype(np.float32)
    wt = s.transpose(2, 3, 1, 0).reshape(9, C, C)   # (k9, i, o)
    wt = wt.reshape(9, NBLK, 128, C)                # (k9, iblk, i, o)
    return wt.astype(ml_dtypes.float8_e4m3), alpha, beta


def make_in_maps(inputs):
    x = np.asarray(inputs['x'], np.float32)
    aa1 = float(np.asarray(inputs['alpha_a1']).reshape(-1)[0])
    aa2 = float(np.asarray(inputs['alpha_a2']).reshape(-1)[0])
    w1t, al1, be1 = _prep_weight(np.asarray(inputs['w1'], np.float32))
    w2t, al2, be2 = _prep_weight(np.asarray(inputs['w2'], np.float32))
    prm = np.zeros((128, NPRM), np.float32)
    f1 = 1.0 / (aa1 * al1)      # z scale relative to the true conv output
    f2 = 1.0 / (aa2 * al2)
    p1 = np.asarray(inputs['pos1'], np.float64)
    n1 = np.asarray(inputs['neg1'], np.float64)
    p2 = np.asarray(inputs['pos2'], np.float64)
    n2 = np.asarray(inputs['neg2'], np.float64)
    cols = ((0, np.asarray(inputs['g1'], np.float64)),
            (2, np.asarray(inputs['b1'], np.float64)),
            (4, (p1 - n1) / 2),
            (6, (p1 + n1) / 2),
            (8, np.asarray(inputs['g2'], np.float64)),
            (10, np.asarray(inputs['b2'], np.float64)),
            (12, (p2 - n2) / 2),
            (14, (p2 + n2) / 2),
            (16, BN_EPS * f1 * f1),
            (18, BN_EPS * f2 * f2),
            (20, be1 / al1),
            (22, be2 / al2),
            (24, p2 - n2),
            (26, n2))
    for base, arr in cols:
        prm[:, base] = arr[:128]
        prm[:, base + 1] = arr[128:]
    in_maps = []
    for c in range(N_CORES):
        in_maps.append({
            'x': np.ascontiguousarray(x[c * BPC:(c + 1) * BPC]),
            'w1t': w1t, 'w2t': w2t, 'prm': prm,
        })
    return in_maps


_CACHE = {}


def kernel(**inputs):
    in_maps = make_in_maps(inputs)
    if 'run' not in _CACHE:
        nc = build()
        _CACHE['nc'] = nc
        _CACHE['run'] = _make_runner(nc)
    outs = _CACHE['run'](in_maps)
    full = np.concatenate([outs[c] for c in range(N_CORES)], axis=0)
    return full.astype(np.float32)


def _make_runner(nc):
    """Build a cached PJRT executable (same path run_bass_kernel_spmd takes
    under axon, via bass2jax) so repeat calls don't re-trace."""
    import jax
    import jax.numpy as jnp
    from jax.sharding import Mesh, PartitionSpec
    from jax.experimental.shard_map import shard_map
    from concourse import bass2jax

    bass2jax.install_neuronx_cc_hook()
    partition_name = (nc.partition_id_tensor.name
                      if nc.partition_id_tensor else None)
    in_names = []
    out_names = []
    out_avals = []
    for alloc in nc.m.functions[0].allocations:
        if not isinstance(alloc, mybir.MemoryLocationSet):
            continue
        name = alloc.memorylocations[0].name
        if alloc.kind == "ExternalInput":
            if name != partition_name:
                in_names.append(name)
        elif alloc.kind == "ExternalOutput":
            shape = tuple(alloc.tensor_shape)
            dtype = mybir.dt.np(alloc.dtype)
            out_names.append(name)
            out_avals.append(jax.core.ShapedArray(shape, dtype))
    n_params = len(in_names)
    all_names = in_names + out_names
    if partition_name is not None:
        all_names = all_names + [partition_name]

    def _body(*args):
        operands = list(args)
        if partition_name is not None:
            operands.append(bass2jax.partition_id_tensor())
        outs = bass2jax._bass_exec_p.bind(
            *operands,
            out_avals=tuple(out_avals),
            in_names=tuple(all_names),
            out_names=tuple(out_names),
            lowering_input_output_aliases=(),
            sim_require_finite=True,
            sim_require_nnan=True,
            nc=nc,
        )
        return tuple(outs)

    devices = jax.devices()[:N_CORES]
    mesh = Mesh(np.asarray(devices), ("core",))
    n_outs = len(out_names)
    sharded = jax.jit(
        shard_map(_body, mesh=mesh,
                  in_specs=(PartitionSpec("core"),) * (n_params + n_outs),
                  out_specs=(PartitionSpec("core"),) * n_outs,
                  check_rep=False),
        donate_argnums=tuple(range(n_params, n_params + n_outs)),
        keep_unused=True,
    )
    sharded_nodonate = jax.jit(
        shard_map(_body, mesh=mesh,
                  in_specs=(PartitionSpec("core"),) * (n_params + n_outs),
                  out_specs=(PartitionSpec("core"),) * n_outs,
                  check_rep=False),
        keep_unused=True,
    )

    def run(in_maps):
        concat_in = [
            np.concatenate([np.asarray(in_maps[c][n]) for c in range(N_CORES)],
                           axis=0)
            for n in in_names
        ]
        concat_zeros = [
            np.zeros((N_CORES * a.shape[0], *a.shape[1:]), a.dtype)
            for a in out_avals
        ]
        out_arrs = sharded(*concat_in, *concat_zeros)
        i = out_names.index("out")
        full = np.asarray(out_arrs[i]).reshape(N_CORES, *out_avals[i].shape)
        return [full[c] for c in range(N_CORES)]

    def stage(in_maps):
        """device_put inputs once; return a dispatch closure for timing."""
        from jax.sharding import NamedSharding
        sh = NamedSharding(mesh, PartitionSpec("core"))
        concat_in = [
            jax.device_put(np.concatenate(
                [np.asarray(in_maps[c][n]) for c in range(N_CORES)], axis=0), sh)
            for n in in_names
        ]
        concat_zeros = [
            jax.device_put(
                np.zeros((N_CORES * a.shape[0], *a.shape[1:]), a.dtype), sh)
            for a in out_avals
        ]

        def dispatch():
            return sharded_nodonate(*concat_in, *concat_zeros)

        return dispatch

    run.stage = stage
    return run
